# revision 2
# baseline (speedup 1.0000x reference)
"""APPNP propagation (10 iterations of h <- 0.9*A@h + 0.1*x) on 8 TRN2 NeuronCores.

Strategy (row sharding + sectioned ELLPACK via dma_gather), v2:
  - Nodes are assigned to (core, quarter, slot, partition) cells. The owner
    core comes from a degree-sorted snake (balances per-core edge counts).
    The quarter q (= table section) is chosen per node by an exponential-
    potential greedy that balances every row's per-section edge counts --
    this cuts the ELLPACK padding from ~2.2x to ~1.3x, which directly cuts
    the GPSIMD descriptor-generation time that dominates the kernel.
  - Within (core, quarter), rows are bin-packed into slots (first-fit
    decreasing on per-section max count, cross-core aware) to minimize the
    per-slot max counts g that the uniform segmented reduce pads to.
  - The gather table holds h in bf16 padded to 256B row stride ([NPAD, 128]
    bf16, cols 0..63 used) because dma_gather requires a 256B-multiple row
    stride. Table rows are quarter-major: row = q*SEC + core*3200 + i, so
    each table section is produced by one quarter-AllGather.
  - Per iteration: per quarter, per chunk-of-slots, per section: idx DMA ->
    dma_gather (Pool/SWDGE) -> weight multiply + segmented reduce (DVE) ->
    per-chunk section combine; after a quarter's chunks finish, its shard
    slice is written and its AllGather (Shared output) fires, overlapping
    the remaining quarters' descriptor generation.
"""

import sys

sys.path.insert(0, "/opt/trn_rl_repo")

import numpy as np
import ml_dtypes

from concourse import bass, bacc, tile, mybir
from concourse import ap_utils
from concourse.bass_utils import run_bass_kernel_spmd

P = 128
D = 64
NCORES = 8
ALPHA = 0.1
K_STEPS = 10
WRAP = 16  # dma_gather index wrap
NQ = 4  # quarters == table sections

LAST_RESULT = None  # test harness reads exec_time_ns from here


class Cfg:
    def __init__(self, n_nodes, nb, ns_chunk):
        self.N = n_nodes
        self.NB = nb  # slots per core (multiple of NQ)
        self.SPQ = nb // NQ  # slots per quarter
        self.SHARD = nb * P
        self.QSH = self.SHARD // NQ  # rows per (core, quarter)
        self.NPAD = NCORES * nb * P
        self.SEC = self.NPAD // NQ  # table section size (int16 index range)
        self.NS = ns_chunk  # max ns*g positions per (chunk, section)
        assert self.SEC <= 32767 + 1


FULL = Cfg(100000, 100, 48)


def dma_gather_128(gp, out_ap, in_ap, idxs_ap, num_idxs, elem_size, elem_step, queue_num=0):
    """nc.gpsimd.dma_gather minus the (transpose-only) elem%256B assert.

    Non-transpose, DRAM source. The Q7 ucode only requires the row *stride*
    (elem_step bytes) to be a multiple of 256.
    """
    assert idxs_ap.dtype == mybir.dt.int16
    assert in_ap.dtype == out_ap.dtype
    stride_bytes = elem_step * mybir.dt.size(in_ap.dtype)
    stride_bytes_256 = stride_bytes // 256
    assert stride_bytes_256 * 256 == stride_bytes and stride_bytes_256 < 256
    assert ap_utils.ap_is_contiguous(out_ap.ap[1:])
    assert ap_utils.ap_is_contiguous(idxs_ap.ap[1:])
    assert in_ap.ap[-1][1] == out_ap.ap[-1][1] == elem_size
    assert out_ap.ap[0][1] * out_ap.ap[1][1] == ((num_idxs + 127) // 128) * 128
    assert in_ap.ap[0][0] == elem_step
    _in_ap = gp.lower_ap_dma(in_ap, for_custom_bir_dma=True)
    _idxs_ap = gp.lower_ap(idxs_ap)
    _out_ap = gp.lower_ap(out_ap)
    return gp.add_instruction(
        mybir.InstDMAGatherAnt(
            name=gp.bass.get_next_instruction_name(),
            ins=[*_in_ap, _idxs_ap, gp.lower_val_access(gp.to_reg(num_idxs))],
            outs=[_out_ap],
            transpose=False,
            num_idxs=num_idxs,
            elem_size=elem_size,
            stride_bytes_256=stride_bytes_256,
            gen_mode=0,
            single_packet=False,
            queue_num=queue_num,
            sbuf_tokens_per_rank=0,
            sbuf_free_dim_per_rank=0,
            sbuf_free_dim_pad_per_rank=0,
            sbuf_byte_offset=0,
        )
    )


def _preprocess(cfg, x, edge_row, edge_col, edge_weight):
    N, NB, SHARD, NPAD = cfg.N, cfg.NB, cfg.SHARD, cfg.NPAD
    SEC, SPQ, QSH, NS = cfg.SEC, cfg.SPQ, cfg.QSH, cfg.NS
    er = edge_row.astype(np.int64)
    ec = edge_col.astype(np.int64)
    deg = np.bincount(er, minlength=N)

    # --- 1. owner core per node: snake over degree-sorted 128-blocks -----
    order = np.argsort(-deg, kind="stable")
    own_of_node = np.empty(N, np.int64)
    nblocks = (N + P - 1) // P
    for b in range(nblocks):
        s = b // NCORES
        j = b % NCORES
        k = j if s % 2 == 0 else NCORES - 1 - j
        own_of_node[order[b * P : (b + 1) * P]] = k

    # --- 2. per-node quarter via exponential-potential greedy ------------
    order_c = np.argsort(ec, kind="stable")
    er_byc = er[order_c]
    starts = np.searchsorted(ec[order_c], np.arange(N + 1))
    outdeg = np.diff(starts)
    cnt = np.zeros((N, NQ), np.float64)
    cap = np.full((NCORES, NQ), QSH, np.int64)
    target = deg / float(NQ)
    q_of = np.full(N, -1, np.int8)
    for n in np.argsort(-outdeg, kind="stable"):
        rows = er_byc[starts[n] : starts[n + 1]]
        k = own_of_node[n]
        sc = np.exp(0.5 * (cnt[rows] - target[rows][:, None])).sum(axis=0)
        sc[cap[k] <= 0] = np.inf
        q = int(np.argmin(sc))
        q_of[n] = q
        cap[k, q] -= 1
        cnt[rows, q] += 1.0

    cnt_i = np.zeros((N, NQ), np.int64)
    np.add.at(cnt_i, (er, q_of[ec].astype(np.int64)), 1)

    # --- 3. smart-pack rows into slots; build cell assignment ------------
    # cell_of_node: node -> (slot s in 0..NB-1 with s//SPQ == q, partition p)
    g_slot = np.zeros((NB, NQ), np.int64)
    s_of_node = np.full(N, -1, np.int64)
    p_of_node = np.full(N, -1, np.int64)
    i_of_node = np.full(N, -1, np.int64)  # index within (core, quarter)
    for k in range(NCORES):
        mine = np.nonzero(own_of_node == k)[0]
        for q in range(NQ):
            nq = mine[q_of[mine] == q]
            m = cnt_i[nq]
            ord2 = np.argsort(-m.max(axis=1), kind="stable")
            nq = nq[ord2]
            m = m[ord2]
            sg = g_slot[SPQ * q : SPQ * (q + 1)].copy()
            capn = np.full(SPQ, P, np.int64)
            fill = (P - capn).copy()
            for i in range(len(nq)):
                inc = np.maximum(sg, m[i]).sum(axis=1) - sg.sum(axis=1)
                inc = np.where(capn > 0, inc, 1 << 40)
                s = int(np.argmin(inc))
                sg[s] = np.maximum(sg[s], m[i])
                node = nq[i]
                s_of_node[node] = SPQ * q + s
                p_of_node[node] = fill[s]
                i_of_node[node] = s * P + fill[s]
                fill[s] += 1
                capn[s] -= 1
            g_slot[SPQ * q : SPQ * (q + 1)] = sg
    g_slot = np.maximum(g_slot, 1)

    # --- 4. chunks of slots within each quarter (ns * max_g <= NS) -------
    chunk_lo, chunk_ns = [], []
    for q in range(NQ):
        s0 = SPQ * q
        while s0 < SPQ * (q + 1):
            m = g_slot[s0].copy()
            s1 = s0 + 1
            while s1 < SPQ * (q + 1):
                m2 = np.maximum(m, g_slot[s1])
                if (s1 + 1 - s0) * m2.max() > NS:
                    break
                m = m2
                s1 += 1
            chunk_lo.append(s0)
            chunk_ns.append(s1 - s0)
            s0 = s1
    nchunk = len(chunk_ns)
    k_of_slot = np.zeros(NB, np.int64)
    sloc_of_slot = np.zeros(NB, np.int64)
    for c in range(nchunk):
        for i in range(chunk_ns[c]):
            k_of_slot[chunk_lo[c] + i] = c
            sloc_of_slot[chunk_lo[c] + i] = i
    g_kc = np.ones((nchunk, NQ), np.int64)
    for c in range(nchunk):
        g_kc[c] = g_slot[chunk_lo[c] : chunk_lo[c] + chunk_ns[c]].max(axis=0)

    base_kc = np.zeros((nchunk, NQ), np.int64)
    tot = 0
    for c in range(nchunk):
        for j in range(NQ):
            base_kc[c, j] = tot
            tot += chunk_ns[c] * g_kc[c, j]

    # --- 5. per-edge placement ------------------------------------------
    # table row of node n
    tab_of_node = (
        q_of.astype(np.int64) * SEC + own_of_node * QSH + i_of_node
    )
    core_e = own_of_node[er]
    s_e = s_of_node[er]
    p_e = p_of_node[er]
    sec_e = q_of[ec].astype(np.int64)
    loc_e = (tab_of_node[ec] - sec_e * SEC).astype(np.int32)
    assert loc_e.min() >= 0 and loc_e.max() < SEC
    w = edge_weight.astype(np.float32)

    # rank of edge within its (row-cell, section) group
    cellid = (core_e * SHARD + s_e * P + p_e) * NQ + sec_e
    eorder = np.argsort(cellid, kind="stable")
    ck_s = cellid[eorder]
    gstarts = np.searchsorted(ck_s, ck_s)
    j_sorted = np.arange(len(ck_s)) - gstarts
    j_e = np.empty(len(ck_s), np.int64)
    j_e[eorder] = j_sorted

    k_e = k_of_slot[s_e]
    sloc_e = sloc_of_slot[s_e]
    assert (j_e < g_kc[k_e, sec_e]).all()
    pos_e = base_kc[k_e, sec_e] + sloc_e * g_kc[k_e, sec_e] + j_e

    idx16 = np.zeros((NCORES, WRAP, tot * P // WRAP), np.int16)
    w_ell = np.zeros((NCORES, P, tot), ml_dtypes.bfloat16)
    i_e = pos_e * P + p_e
    for k in range(NCORES):
        m = core_e == k
        w_ell[k][p_e[m], pos_e[m]] = w[m].astype(ml_dtypes.bfloat16)
        ii = i_e[m]
        idx16[k][ii % WRAP, ii // WRAP] = loc_e[m]
    idxw = np.ascontiguousarray(np.tile(idx16, (NCORES, 1)))  # [NCORES, 128, tot*8]

    # --- 6. x / h0 / unpermute maps -------------------------------------
    # row-cell global index (core*SHARD + s*P + p) -> node
    new_rows_old = np.full(NPAD, -1, np.int64)
    cell_g = own_of_node * SHARD + s_of_node * P + p_of_node
    new_rows_old[cell_g] = np.arange(N)

    scale = ALPHA / (1.0 - ALPHA)
    x_ell = np.zeros((NCORES, P, NB, D), np.float32)
    x_new = np.zeros((NPAD, D), np.float32)
    mask = new_rows_old >= 0
    x_new[mask] = x[new_rows_old[mask]]
    for k in range(NCORES):
        x_ell[k] = (
            (x_new[k * SHARD : (k + 1) * SHARD] * scale)
            .reshape(NB, P, D)
            .transpose(1, 0, 2)
        )

    h0 = np.zeros((NPAD, 2 * D), ml_dtypes.bfloat16)
    h0[tab_of_node, :D] = x.astype(ml_dtypes.bfloat16)

    struct = (
        tuple(chunk_ns),
        tuple(tuple(int(g) for g in row) for row in g_kc),
        tuple(tuple(int(b) for b in row) for row in base_kc),
        int(tot),
        tuple(int(v) for v in chunk_lo),
    )
    return struct, idxw, w_ell, x_ell, h0, new_rows_old


def _build(cfg, struct):
    chunk_ns, g_kc, base_kc, tot, chunk_lo = struct
    NB, SHARD, NPAD, SEC = cfg.NB, cfg.SHARD, cfg.NPAD, cfg.SEC
    SPQ, QSH = cfg.SPQ, cfg.QSH
    nchunk = len(chunk_ns)
    max_npos = max(
        chunk_ns[k] * g_kc[k][c] for k in range(nchunk) for c in range(NQ)
    )
    max_ns = max(chunk_ns)

    nc = bacc.Bacc("TRN2", target_bir_lowering=False, debug=False, num_devices=NCORES, num_swdge_queues=4)
    bf16, f32, i16 = mybir.dt.bfloat16, mybir.dt.float32, mybir.dt.int16

    idxw_in = nc.dram_tensor("idxw", [P, tot * 8], i16, kind="ExternalInput")
    w_in = nc.dram_tensor("w", [P, tot], bf16, kind="ExternalInput")
    x_in = nc.dram_tensor("x", [P, NB, D], f32, kind="ExternalInput")
    h0_in = nc.dram_tensor("h0", [NPAD, 2 * D], bf16, kind="ExternalInput")
    out_ext = nc.dram_tensor("out", [P, NB, D], f32, kind="ExternalOutput")

    tabA = nc.dram_tensor("tabA", [NPAD, 2 * D], bf16, addr_space="Shared")
    tabB = nc.dram_tensor("tabB", [NPAD, 2 * D], bf16, addr_space="Shared")
    sbA = nc.dram_tensor("sbA", [SHARD, 2 * D], bf16)
    sbB = nc.dram_tensor("sbB", [SHARD, 2 * D], bf16)

    # chunks grouped per quarter
    chunks_of_q = [[] for _ in range(NQ)]
    for c in range(nchunk):
        chunks_of_q[chunk_lo[c] // SPQ].append(c)

    with tile.TileContext(nc) as tc:
        with (
            tc.tile_pool(name="const", bufs=1) as cpool,
            tc.tile_pool(name="ix", bufs=6) as ipool,
            tc.tile_pool(name="gath", bufs=6) as gpool,
            tc.tile_pool(name="pp", bufs=2) as ppool,
            tc.tile_pool(name="red", bufs=1) as rpool,
        ):
            w_t = cpool.tile([P, tot], bf16, tag="w")
            x_t = cpool.tile([P, NB * D], f32, tag="x")
            hb2 = cpool.tile([P, NB * 2 * D], bf16, tag="hb2")
            nc.sync.dma_start(out=w_t[:], in_=w_in[:])
            nc.sync.dma_start(out=x_t[:], in_=x_in[:].rearrange("p b d -> p (b d)"))
            hb2v = hb2[:].rearrange("p (s e) -> p s e", e=2 * D)
            nc.gpsimd.memset(hb2v[:, :, D:], 0)

            tables = [h0_in]
            for t in range(K_STEPS - 1):
                tables.append(tabA if t % 2 == 0 else tabB)

            gq = 0  # round-robin gather queue
            for t in range(K_STEPS):
                src = tables[t]
                red = rpool.tile([P, NB * D], f32, tag="red")
                sb = sbA if t % 2 == 0 else sbB
                for q in range(NQ):
                    for k in chunks_of_q[q]:
                        ns = chunk_ns[k]
                        pp = ppool.tile([P, max_ns * NQ * D], f32, tag="pp")
                        for c in range(NQ):
                            g = g_kc[k][c]
                            npos = ns * g
                            lo = base_kc[k][c]
                            it = ipool.tile([P, max_npos * 8], i16, tag="ix")
                            nc.sync.dma_start(
                                out=it[:, : npos * 8],
                                in_=idxw_in[:, lo * 8 : (lo + npos) * 8],
                            )
                            gt = gpool.tile([P, max_npos * D], bf16, tag="g")
                            dma_gather_128(
                                nc.gpsimd,
                                out_ap=gt[:, : npos * D].rearrange("p (n d) -> p n d", d=D),
                                in_ap=src[c * SEC : (c + 1) * SEC, :D],
                                idxs_ap=it[:, : npos * 8],
                                num_idxs=npos * P,
                                elem_size=D,
                                elem_step=2 * D,
                                queue_num=gq % 4,
                            )
                            gq += 1
                            wb = w_t[:, lo : lo + npos].unsqueeze(-1).to_broadcast([P, npos, D])
                            nc.vector.tensor_tensor(
                                out=gt[:, : npos * D].rearrange("p (n d) -> p n d", d=D),
                                in0=gt[:, : npos * D].rearrange("p (n d) -> p n d", d=D),
                                in1=wb,
                                op=mybir.AluOpType.mult,
                            )
                            # reduce over j: [P, ns, D, g] -> pp[:, :, c, :]
                            seg = gt[:, : npos * D].rearrange("p (s g d) -> p s d g", g=g, d=D)
                            ppv = pp[:, : ns * NQ * D].rearrange(
                                "p (s c d) -> p s c d", c=NQ, d=D
                            )[:, :, c, :]
                            nc.vector.tensor_reduce(
                                out=ppv, in_=seg, axis=mybir.AxisListType.X, op=mybir.AluOpType.add
                            )
                        # combine sections: [P, ns, D, NQ] -> red slot range
                        s0 = chunk_lo[k]
                        nc.vector.tensor_reduce(
                            out=red[:, s0 * D : (s0 + ns) * D],
                            in_=pp[:, : ns * NQ * D].rearrange(
                                "p (s c d) -> p s d c", c=NQ, d=D
                            ),
                            axis=mybir.AxisListType.X,
                            op=mybir.AluOpType.add,
                        )
                    if t < K_STEPS - 1:
                        # quarter q done: residual + scale -> bf16, ship shard slice
                        sl = slice(SPQ * q * D, SPQ * (q + 1) * D)
                        nc.vector.tensor_tensor(
                            out=red[:, sl],
                            in0=red[:, sl],
                            in1=x_t[:, sl],
                            op=mybir.AluOpType.add,
                        )
                        nc.vector.tensor_scalar_mul(
                            out=hb2v[:, SPQ * q : SPQ * (q + 1), :D],
                            in0=red[:, sl].rearrange("p (s d) -> p s d", d=D),
                            scalar1=1.0 - ALPHA,
                        )
                        nc.sync.dma_start(
                            out=sb[QSH * q : QSH * (q + 1), :].rearrange(
                                "(s p) e -> p s e", p=P
                            ),
                            in_=hb2v[:, SPQ * q : SPQ * (q + 1), :],
                        )
                        nc.gpsimd.collective_compute(
                            "AllGather",
                            mybir.AluOpType.bypass,
                            replica_groups=[list(range(NCORES))],
                            ins=[sb[QSH * q : QSH * (q + 1), :].opt()],
                            outs=[tables[t + 1][SEC * q : SEC * (q + 1), :].opt()],
                        )
                if t == K_STEPS - 1:
                    nc.vector.tensor_tensor(
                        out=red[:], in0=red[:], in1=x_t[:], op=mybir.AluOpType.add
                    )
                    nc.vector.tensor_scalar_mul(out=red[:], in0=red[:], scalar1=1.0 - ALPHA)
                    nc.sync.dma_start(
                        out=out_ext[:].rearrange("p b d -> p (b d)"), in_=red[:]
                    )
    nc.compile()
    return nc


_BUILD_CACHE = {}


def _kernel_impl(cfg, x, edge_row, edge_col, edge_weight, trace=False):
    global LAST_RESULT
    struct, idxw, w_ell, x_ell, h0, new_rows_old = _preprocess(
        cfg, x, edge_row, edge_col, edge_weight
    )
    key = (cfg.N, struct[0], struct[1])
    if key not in _BUILD_CACHE:
        _BUILD_CACHE[key] = _build(cfg, struct)
    nc = _BUILD_CACHE[key]

    in_maps = [
        {"idxw": idxw[k], "w": w_ell[k], "x": x_ell[k], "h0": h0}
        for k in range(NCORES)
    ]
    res = run_bass_kernel_spmd(nc, in_maps, core_ids=list(range(NCORES)), trace=trace)
    LAST_RESULT = res

    SHARD = cfg.SHARD
    full_new = np.empty((cfg.NPAD, D), np.float32)
    for k in range(NCORES):
        o = np.asarray(res.results[k]["out"]).reshape(P, cfg.NB, D)
        full_new[k * SHARD : (k + 1) * SHARD] = o.transpose(1, 0, 2).reshape(SHARD, D)
    out = np.empty((cfg.N, D), np.float32)
    mask = new_rows_old >= 0
    out[new_rows_old[mask]] = full_new[mask]
    return out


def kernel(x, edge_row, edge_col, edge_weight, _trace=False):
    x = np.asarray(x, dtype=np.float32)
    edge_row = np.asarray(edge_row, dtype=np.int32)
    edge_col = np.asarray(edge_col, dtype=np.int32)
    edge_weight = np.asarray(edge_weight, dtype=np.float32)
    return _kernel_impl(FULL, x, edge_row, edge_col, edge_weight, trace=_trace)
